# revision 7
# baseline (speedup 1.0000x reference)
"""NT-Xent contrastive loss kernel for TRN2, 8 NeuronCores.

Problem: z_i, z_j [4096, 256] f32.
  reps = concat(l2norm(z_i), l2norm(z_j))      # [8192, 256]
  sim  = reps @ reps.T                          # [8192, 8192]
  sim  = where(eye, -1e9, sim) / 0.07
  positives[i] = sim[i, (i+4096) % 8192]
  loss = mean(-(positives - log(sum(exp(sim), axis=1))))

Sharding: rows of sim split across 8 cores (1024 rows each). Each core
receives the full rep matrix with its rows ROTATED to the front
(np.roll by -core*1024), so one static SPMD program works for all
cores: local row r is global row core*1024+r, its self-match is at
local column r and its positive at local column r+4096.

Per core:
  1. Normalize all 8192 rows on-device (sumsq via DVE fused
     multiply-reduce, inv-norm = exp(-0.5*ln(sumsq)) on ACT), scale rows,
     transpose via PE into repsT [256part, 8192] held as 8 SBUF tiles.
  2. For each of 8 row-tiles x 4 col-chunks [128, 2048]: fp32r matmuls
     into PSUM, zero the self-diagonal block (multiply by 1-I), extract
     the positive diagonal (multiply by I with fused row-reduce), then
     one ACT Exp over the chunk with fused row-sum (accum_out).
  3. denom = sum(partials) - 1 (the zeroed diag contributes exp(0)=1);
     per-row loss = ln(denom) - pos/T; DMA out [128, 8].
Host sums the 8x1024 per-row losses and divides by 8192.
"""

import sys

import numpy as np

for _p in ("/opt/trn_rl_repo", "/root/.axon_site/_ro/trn_rl_repo"):
    if _p not in sys.path:
        sys.path.append(_p)

B = 4096
D = 256
N2 = 2 * B                  # 8192 total rows
NCORES = 8
ROWS_PER_CORE = N2 // NCORES    # 1024
RT = ROWS_PER_CORE // 128       # 8 row-tiles per core
CHUNK = 2048                    # psum col-chunk (4 banks)
NCHUNK = N2 // CHUNK            # 4
NGROUP = 4                      # repsT col groups of 2048 rows
TILES_PER_GROUP = CHUNK // 128  # 16
TEMP = 0.07
INV_T = 1.0 / TEMP

_CACHE = {}


def _build_nc():
    import concourse.bacc as bacc
    import concourse.mybir as mybir
    import concourse.tile as tile
    from contextlib import ExitStack

    f32 = mybir.dt.float32
    f32r = mybir.dt.float32r
    Act = mybir.ActivationFunctionType
    Alu = mybir.AluOpType
    Ax = mybir.AxisListType

    nc = bacc.Bacc("TRN2", target_bir_lowering=False, debug=False)

    reps_d = nc.dram_tensor("reps", [N2, D], f32, kind="ExternalInput").ap()
    eye_d = nc.dram_tensor("eye", [128, 128], f32, kind="ExternalInput").ap()
    ome_d = nc.dram_tensor("ome", [128, 128], f32, kind="ExternalInput").ap()
    lout_d = nc.dram_tensor("lout", [128, RT], f32, kind="ExternalOutput").ap()

    with tile.TileContext(nc) as tc, ExitStack() as ctx:
        const_pool = ctx.enter_context(tc.tile_pool(name="const", bufs=1))
        x_pool = ctx.enter_context(tc.tile_pool(name="x", bufs=20))
        xn_pool = ctx.enter_context(tc.tile_pool(name="xn", bufs=6))
        stat_pool = ctx.enter_context(tc.tile_pool(name="stat", bufs=2))
        repsT_pool = ctx.enter_context(tc.tile_pool(name="repsT", bufs=1))
        small_pool = ctx.enter_context(tc.tile_pool(name="small", bufs=1))
        junk_pool = ctx.enter_context(tc.tile_pool(name="junk", bufs=2))
        psum_pool = ctx.enter_context(tc.tile_pool(name="psum", bufs=2, space="PSUM"))

        eye_sb = const_pool.tile([128, 128], f32, tag="eye", name="eye")
        nc.sync.dma_start(eye_sb[:], eye_d)
        ome_sb = const_pool.tile([128, 128], f32, tag="ome", name="ome")
        nc.sync.dma_start(ome_sb[:], ome_d)

        # repsT[h][g]: [128 (d-half h), 2048 (rows of group g)]
        repsT = [
            [repsT_pool.tile([128, CHUNK], f32r, tag=f"repsT{h}_{g}", name=f"repsT{h}_{g}") for g in range(NGROUP)]
            for h in range(2)
        ]

        # accumulators living across the whole main loop
        partials = small_pool.tile([128, RT * NCHUNK], f32, tag="partials", name="partials")
        pos_all = small_pool.tile([128, RT], f32, tag="pos", name="pos")
        den_all = small_pool.tile([128, RT], f32, tag="den", name="den")
        logden = small_pool.tile([128, RT], f32, tag="logden", name="logden")
        loss_t = small_pool.tile([128, RT], f32, tag="loss", name="loss")
        junk128 = small_pool.tile([128, 128], f32, tag="junk128", name="junk128")

        def prologue_group(g):
            """Load rows [g*2048, (g+1)*2048), normalize, transpose into repsT[:][g]."""
            xs = []
            ss = stat_pool.tile([128, TILES_PER_GROUP], f32, tag="ss", name="ss")
            for jl in range(TILES_PER_GROUP):
                j = g * TILES_PER_GROUP + jl
                x = x_pool.tile([128, D], f32, tag="x", name="x")
                nc.sync.dma_start(x[:], reps_d[j * 128:(j + 1) * 128, :])
                xs.append(x)
                xsq = junk_pool.tile([128, D], f32, tag="xsq", name="xsq")
                # xsq = x*x ; ss[:, jl] = sum(xsq)
                nc.vector.scalar_tensor_tensor(
                    out=xsq[:], in0=x[:], scalar=1.0, in1=x[:],
                    op0=Alu.mult, op1=Alu.mult, accum_out=ss[:, jl:jl + 1],
                )
            lns = stat_pool.tile([128, TILES_PER_GROUP], f32, tag="lns", name="lns")
            nc.scalar.activation(lns[:], ss[:], Act.Ln)
            inv = stat_pool.tile([128, TILES_PER_GROUP], f32, tag="inv", name="inv")
            nc.scalar.activation(inv[:], lns[:], Act.Exp, scale=-0.5)

            psg = [psum_pool.tile([128, CHUNK], f32, tag="ps", name="ps") for _ in range(2)]
            for jl in range(TILES_PER_GROUP):
                xn = xn_pool.tile([128, D], f32, tag="xn", name="xn")
                nc.vector.tensor_scalar_mul(xn[:], xs[jl][:], inv[:, jl:jl + 1])
                for h in range(2):
                    nc.tensor.transpose(
                        psg[h][:, jl * 128:(jl + 1) * 128],
                        xn[:, h * 128:(h + 1) * 128],
                        eye_sb[:],
                    )
            for h in range(2):
                nc.vector.tensor_copy(repsT[h][g][:], psg[h][:])

        def main_chunk(c):
            """sim rows (all 8 row-tiles) x cols [c*2048, (c+1)*2048)."""
            for t in range(RT):
                ps = psum_pool.tile([128, CHUNK], f32, tag="ps", name="ps")
                for h in range(2):
                    lhsT = repsT[h][0][:, t * 128:(t + 1) * 128]
                    for b in range(CHUNK // 512):
                        nc.tensor.matmul(
                            ps[:, b * 512:(b + 1) * 512],
                            lhsT,
                            repsT[h][c][:, b * 512:(b + 1) * 512],
                            start=(h == 0),
                            stop=(h == 1),
                        )
                if c == 0:
                    # zero the self-similarity diagonal (block at cols t*128)
                    nc.vector.tensor_mul(
                        ps[:, t * 128:(t + 1) * 128],
                        ps[:, t * 128:(t + 1) * 128],
                        ome_sb[:],
                    )
                if c == 2:
                    # positive pair diagonal (global col 4096 + t*128 + p)
                    nc.vector.scalar_tensor_tensor(
                        out=junk128[:],
                        in0=ps[:, t * 128:(t + 1) * 128], scalar=1.0,
                        in1=eye_sb[:], op0=Alu.mult, op1=Alu.mult,
                        accum_out=pos_all[:, t:t + 1],
                    )
                # exp(sim/T) in place + fused row-sum
                nc.scalar.activation(
                    ps[:], ps[:], Act.Exp, scale=INV_T,
                    accum_out=partials[:, t * NCHUNK + c:t * NCHUNK + c + 1],
                )

        for g in range(NGROUP):
            prologue_group(g)
            main_chunk(g)

        # denom = sum_c partials - 1 (zeroed diag contributed exp(0)=1)
        nc.vector.reduce_sum(
            den_all[:], partials[:].rearrange("p (t c) -> p t c", c=NCHUNK), axis=Ax.X
        )
        nc.vector.tensor_scalar_add(den_all[:], den_all[:], -1.0)
        nc.scalar.activation(logden[:], den_all[:], Act.Ln)
        # loss = ln(denom) - pos/T
        nc.vector.scalar_tensor_tensor(
            out=loss_t[:], in0=pos_all[:], scalar=-INV_T, in1=logden[:],
            op0=Alu.mult, op1=Alu.add,
        )
        nc.sync.dma_start(lout_d, loss_t[:])

    nc.finalize()
    return nc


def _get_nc():
    if "nc" not in _CACHE:
        _CACHE["nc"] = _build_nc()
    return _CACHE["nc"]


def _in_maps(z_i, z_j):
    reps = np.concatenate(
        [np.asarray(z_i, np.float32), np.asarray(z_j, np.float32)], axis=0
    )
    eye = np.eye(128, dtype=np.float32)
    ome = (1.0 - eye).astype(np.float32)
    maps = []
    for m in range(NCORES):
        rot = np.ascontiguousarray(np.roll(reps, -m * ROWS_PER_CORE, axis=0))
        maps.append({"reps": rot, "eye": eye, "ome": ome})
    return maps


def kernel(z_i, z_j):
    from concourse.bass_utils import run_bass_kernel_spmd

    nc = _get_nc()
    res = run_bass_kernel_spmd(nc, _in_maps(z_i, z_j), list(range(NCORES)))
    total = 0.0
    for r in res.results:
        total += float(np.sum(r["lout"], dtype=np.float64))
    return np.float32(total / N2)
